# revision 1
# baseline (speedup 1.0000x reference)
"""Trainium2 Bass kernel for a GNN message-passing layer (GCL).

reference:
    m   = relu(concat(h[row], h[col]) @ edge_w + edge_b)       # [E, H]
    agg = segment_sum(m, row, N)                               # [N, H]
    out = relu(concat(h, agg) @ node_w + node_b)               # [N, H]

Strategy (8 cores, edge-parallel with node-range ownership, no collectives):
  * Precompute AB[n] = [h[n] @ Wtop + b | h[n] @ Wbot]  (bf16, DRAM table).
    Then m_e = relu(AB[row_e].A + AB[col_e].B), a pure gather + add.
  * Sort each core's edges by (col-chunk, dest-window); the dma_gather
    int16-index limit is handled by splitting the col table into 4 chunks.
  * Segment-sum via one-hot matmul: for each 128-edge chunk belonging to one
    128-node window, onehot[e, j] = (row_local[e] == j) built with a DVE
    is_equal against an iota; PE accumulates m.T @ onehot in PSUM.
  * Node MLP with bias folded in via an appended ones-row (K=65 matmul).
Each core owns a contiguous 12544-node range; rows of its edges fall in that
range, so aggregation and the node MLP are fully local.
"""

import math
import numpy as np
import ml_dtypes

import concourse.bass as bass
import concourse.bacc as bacc
import concourse.tile as tile
from concourse import mybir
from concourse.tile import TileContext
from concourse.library_config import mlp as mlp_library

BF16 = mybir.dt.bfloat16
F32 = mybir.dt.float32
I16 = mybir.dt.int16
NP_BF16 = ml_dtypes.bfloat16


class Cfg:
    def __init__(self, n_nodes, n_cores=8, spc=None, table_f32=False):
        self.n_swdge_queues = 4   # parallel SWDGE queues: 8x gather throughput
        self.col_sort = False
        self.N = n_nodes
        self.n_cores = n_cores
        self.NPC = int(math.ceil(n_nodes / n_cores / 128)) * 128
        self.NP = self.NPC * n_cores
        self.W = self.NPC // 128          # windows per core
        self.C = 4                        # col chunks
        assert self.NP % self.C == 0
        self.CHUNK = self.NP // self.C
        assert self.CHUNK <= 32767, "int16 gather index limit"
        if spc is None:
            spc = max(d for d in range(1, 17) if self.W % d == 0 and (self.W // d) % 2 == 0 or d == 1)
        # segments (windows) per gather call; must divide W
        self.SPC = spc
        assert self.W % self.SPC == 0
        self.CALLS_PER_CHUNK = self.W // self.SPC
        # idx loads cover IDX_CALLS gather calls each
        self.IDX_CALLS = self.CALLS_PER_CHUNK // 2 if self.CALLS_PER_CHUNK % 2 == 0 else self.CALLS_PER_CHUNK
        self.table_f32 = table_f32
        self.SEG = None  # set from data

    def stripe(self, total):
        for cand in (8192, 6272, 4096, 3136, 2048, 1792, 1568, 1024, 896, 784, 512, 448, 256, 128):
            if cand <= total and total % cand == 0:
                return cand
        raise AssertionError(total)


def build_kernel(cfg, phases=(0, 1, 2), p1_level=4, p2_level=3):
    """Build the single-core SPMD program. Returns nc.
    p1_level: 1=gathers only, 2=+add/relu, 3=+onehot, 4=full (matmul+flush)."""
    SEG = cfg.SEG
    assert SEG is not None and SEG % 128 == 0
    EP = cfg.C * cfg.W * SEG               # padded edges per core
    NCALL = cfg.SPC * SEG                  # idxs per gather call
    JPC = NCALL // 128                     # 128-chunks per call
    JPS = SEG // 128                       # 128-chunks per segment
    TDT = F32 if cfg.table_f32 else BF16   # gather table dtype
    TESZ = 64 if cfg.table_f32 else 128    # gather elem_size (=256B either way)
    # >64 descriptors/engine in one packet wedges the device; the per-engine
    # descriptor count is NCALL//16 + 1.
    SINGLE_PACKET = (NCALL // 16 + 1) <= 64

    NSWQ = getattr(cfg, "n_swdge_queues", 1)
    nc = bacc.Bacc("TRN2", target_bir_lowering=False, debug=False,
                   num_swdge_queues=NSWQ)

    # ---- DRAM I/O ----
    hTa_d = nc.dram_tensor("hTa", [65, cfg.NP], F32, kind="ExternalInput")
    hTown_d = nc.dram_tensor("hTown", [65, cfg.NPC], F32, kind="ExternalInput")
    waug_d = nc.dram_tensor("waug", [65, 128], F32, kind="ExternalInput")
    nw1_d = nc.dram_tensor("nw1", [64, 64], F32, kind="ExternalInput")
    nw2a_d = nc.dram_tensor("nw2a", [65, 64], F32, kind="ExternalInput")
    iota_d = nc.dram_tensor("iota", [128, 128], BF16, kind="ExternalInput")
    colidx_d = nc.dram_tensor("colidx", [128, EP // 16], I16, kind="ExternalInput")
    rowidx_d = nc.dram_tensor("rowidx", [128, EP // 16], I16, kind="ExternalInput")
    rl_d = nc.dram_tensor("rl", [128, EP // 128], BF16, kind="ExternalInput")
    AB_ds = [nc.dram_tensor(f"AB{c}", [cfg.CHUNK, 128], TDT)
             for c in range(cfg.C)]
    Aown_d = nc.dram_tensor("Aown", [cfg.NPC, 128], TDT)
    out_d = nc.dram_tensor("out", [cfg.NPC, 64], F32, kind="ExternalOutput")

    with TileContext(nc) as tc:
        nc.gpsimd.load_library(mlp_library)

        with tc.tile_pool(name="const", bufs=1) as cpool:
            waug_sb = cpool.tile([65, 128], F32)
            nc.sync.dma_start(out=waug_sb[:], in_=waug_d[:])
            iota_sb = cpool.tile([128, 128], BF16)
            nc.sync.dma_start(out=iota_sb[:], in_=iota_d[:])
            nw1_sb = cpool.tile([64, 64], F32)
            nc.sync.dma_start(out=nw1_sb[:], in_=nw1_d[:])
            nw2a_sb = cpool.tile([65, 64], F32)
            nc.sync.dma_start(out=nw2a_sb[:], in_=nw2a_d[:])

            # aggT arena [65, NPC]: rows 0:64 = aggT, row 64 = ones (bias row)
            arena = cpool.tile([65, cfg.NPC], F32)
            nc.vector.memset(arena[64:65, :], 1.0)

            # ---- Phase 0: build AB table (all NP nodes) and Aown (own nodes) ----
            def ab_pass(src_d, dst, total):
                SN = cfg.stripe(cfg.CHUNK if isinstance(dst, list) else total)
                JT = SN // 128
                with tc.tile_pool(name="p0", bufs=2) as p0, \
                     tc.tile_pool(name="p0ps", bufs=4, space="PSUM") as p0ps:
                    for s in range(total // SN):
                        hstripe = p0.tile([65, SN], F32, tag="hstripe")
                        nc.sync.dma_start(
                            out=hstripe[:], in_=src_d[:, s * SN:(s + 1) * SN])
                        abst = p0.tile([128, JT, TESZ * (2 if cfg.table_f32 else 1)], TDT, tag="abst")
                        for j in range(JT):
                            ps = p0ps.tile([128, 128], F32)
                            nc.tensor.matmul(
                                out=ps[:], lhsT=hstripe[:, j * 128:(j + 1) * 128],
                                rhs=waug_sb[:], start=True, stop=True)
                            nc.vector.tensor_copy(out=abst[:, j, :], in_=ps[:])
                        if isinstance(dst, list):
                            n0 = s * SN
                            dst_d, off = dst[n0 // cfg.CHUNK], n0 % cfg.CHUNK
                        else:
                            dst_d, off = dst, s * SN
                        nc.sync.dma_start(
                            out=dst_d[off:off + SN, :].rearrange(
                                "(j p) f -> p j f", p=128),
                            in_=abst[:])

            if 0 in phases:
                ab_pass(hTown_d, Aown_d, cfg.NPC)
                ab_pass(hTa_d, AB_ds, cfg.NP)

            # ---- Phase 1: gather + edge MLP + one-hot aggregation ----
            if 1 in phases:
              with tc.tile_pool(name="rlp", bufs=1) as rlp:
                rl_sb = rlp.tile([128, EP // 128], BF16)
                nc.sync.dma_start(out=rl_sb[:], in_=rl_d[:])

                with tc.tile_pool(name="idxp", bufs=2) as idxp, \
                     tc.tile_pool(name="gath", bufs=4) as gathp, \
                     tc.tile_pool(name="mp", bufs=3) as mp, \
                     tc.tile_pool(name="ohp", bufs=3) as ohp, \
                     tc.tile_pool(name="p1ps", bufs=4, space="PSUM") as p1ps:
                    IC = cfg.IDX_CALLS
                    ILEN = IC * NCALL // 16      # idx cols per load
                    for c in range(cfg.C):
                        col_tab = AB_ds[c][:, 64:128] if cfg.table_f32 \
                            else AB_ds[c][:]
                        row_tab = Aown_d[:, 0:64] if cfg.table_f32 else Aown_d[:]
                        for g in range(cfg.CALLS_PER_CHUNK // IC):
                            goff = (c * cfg.CALLS_PER_CHUNK + g * IC) * NCALL // 16
                            cidx = idxp.tile([128, ILEN], I16, tag="cidx")
                            nc.sync.dma_start(
                                out=cidx[:], in_=colidx_d[:, goff:goff + ILEN])
                            ridx = idxp.tile([128, ILEN], I16, tag="ridx")
                            nc.sync.dma_start(
                                out=ridx[:], in_=rowidx_d[:, goff:goff + ILEN])
                            for cl in range(IC):
                                call = (c * cfg.CALLS_PER_CHUNK + g * IC + cl)
                                colg = gathp.tile([128, JPC, TESZ], TDT, tag="g")
                                nc.gpsimd.dma_gather(
                                    colg[:], col_tab,
                                    cidx[:, cl * (NCALL // 16):(cl + 1) * (NCALL // 16)],
                                    NCALL, NCALL, TESZ, single_packet=SINGLE_PACKET,
                                    queue_num=(2 * call) % NSWQ,
                                    elem_step=128 if cfg.table_f32 else None)
                                rowg = gathp.tile([128, JPC, TESZ], TDT, tag="g")
                                nc.gpsimd.dma_gather(
                                    rowg[:], row_tab,
                                    ridx[:, cl * (NCALL // 16):(cl + 1) * (NCALL // 16)],
                                    NCALL, NCALL, TESZ, single_packet=SINGLE_PACKET,
                                    queue_num=(2 * call + 1) % NSWQ,
                                    elem_step=128 if cfg.table_f32 else None)
                                for s in range(cfg.SPC if p1_level >= 2 else 0):
                                    w = (call % cfg.CALLS_PER_CHUNK) * cfg.SPC + s
                                    gseg = call * cfg.SPC + s
                                    j0 = s * JPS
                                    a_sl = rowg[:, j0:j0 + JPS, 0:64]
                                    b_sl = colg[:, j0:j0 + JPS, 64:128] if not cfg.table_f32 \
                                        else colg[:, j0:j0 + JPS, 0:64]
                                    madd = mp.tile([128, JPS, 64], TDT, tag="madd")
                                    nc.vector.tensor_tensor(
                                        out=madd[:], in0=a_sl, in1=b_sl,
                                        op=mybir.AluOpType.add)
                                    m2 = mp.tile([128, JPS, 64], TDT, tag="m2")
                                    nc.scalar.activation(
                                        out=m2[:], in_=madd[:],
                                        func=mybir.ActivationFunctionType.Relu)
                                    if p1_level < 3:
                                        continue
                                    oh = ohp.tile([128, JPS, 128], TDT, tag="oh")
                                    nc.vector.tensor_tensor(
                                        out=oh[:],
                                        in0=rl_sb[:, gseg * JPS:gseg * JPS + JPS]
                                            .to_broadcast([128, JPS, 128]),
                                        in1=iota_sb[:].rearrange("p (a b) -> p a b", a=1)
                                            .to_broadcast([128, JPS, 128]),
                                        op=mybir.AluOpType.is_equal)
                                    if p1_level < 4:
                                        continue
                                    ps = p1ps.tile([64, 128], F32)
                                    for j in range(JPS):
                                        nc.tensor.matmul(
                                            out=ps[:], lhsT=m2[:, j, :], rhs=oh[:, j, :],
                                            start=(j == 0), stop=(j == JPS - 1))
                                    dst = arena[0:64, w * 128:(w + 1) * 128]
                                    if c == 0:
                                        nc.vector.tensor_copy(out=dst, in_=ps[:])
                                    else:
                                        nc.vector.tensor_tensor(
                                            out=dst, in0=dst, in1=ps[:],
                                            op=mybir.AluOpType.add)

            # ---- Phase 2: node MLP ----
            if 2 in phases:
              with tc.tile_pool(name="p2", bufs=2) as p2, \
                 tc.tile_pool(name="p2ps", bufs=4, space="PSUM") as p2ps:
                GW = cfg.SPC                  # windows per output group
                for g in range(cfg.W // GW):
                    hT2 = p2.tile([65, GW * 128], F32, tag="hT2")
                    nc.sync.dma_start(
                        out=hT2[:],
                        in_=hTown_d[:, g * GW * 128:(g + 1) * GW * 128])
                    ost = p2.tile([128, GW, 64], F32, tag="ost")
                    for i in range(GW):
                        w = g * GW + i
                        if p2_level < 2:
                            nc.vector.memset(ost[:, i, :], 0.0)
                            continue
                        ps = p2ps.tile([128, 64], F32)
                        nc.tensor.matmul(
                            out=ps[:], lhsT=hT2[0:64, i * 128:(i + 1) * 128],
                            rhs=nw1_sb[:], start=True, stop=False)
                        nc.tensor.matmul(
                            out=ps[:], lhsT=arena[:, w * 128:(w + 1) * 128],
                            rhs=nw2a_sb[:], start=False, stop=True)
                        if p2_level < 3:
                            nc.vector.tensor_copy(out=ost[:, i, :], in_=ps[:])
                        else:
                            nc.scalar.activation(
                                out=ost[:, i, :], in_=ps[:],
                                func=mybir.ActivationFunctionType.Relu)
                    nc.sync.dma_start(
                        out=out_d[g * GW * 128:(g + 1) * GW * 128, :].rearrange(
                            "(j p) f -> p j f", p=128),
                        in_=ost[:])

    nc.compile()
    return nc


# ---------------- host-side data prep ----------------

def _wrap16(a):
    x = np.ascontiguousarray(a.reshape(-1, 16).T)
    return np.tile(x, (8, 1))


def _wrap128(a):
    return np.ascontiguousarray(a.reshape(-1, 128).T)


def prep_inputs(cfg, h, edge_index, edge_w, edge_b, node_w, node_b):
    """Returns (in_maps, SEG). Sets cfg.SEG."""
    N = cfg.N
    row = np.asarray(edge_index[0])
    col = np.asarray(edge_index[1])
    h = np.asarray(h, dtype=np.float32)

    # hT augmented with ones row, padded to NP cols
    hTa = np.zeros((65, cfg.NP), np.float32)
    hTa[:64, :N] = h.T
    hTa[64, :] = 1.0

    waug = np.zeros((65, 128), np.float32)
    waug[:64, 0:64] = edge_w[:64]
    waug[:64, 64:128] = edge_w[64:]
    waug[64, 0:64] = edge_b

    nw1 = np.ascontiguousarray(node_w[:64], dtype=np.float32)
    nw2a = np.concatenate([node_w[64:], node_b[None, :]], axis=0).astype(np.float32)

    iota = np.tile(np.arange(128, dtype=np.float32), (128, 1)).astype(NP_BF16)

    # per-core edge prep; SEG = global max segment length (uniform program)
    per_core = []
    maxc = 1
    for k in range(cfg.n_cores):
        base = k * cfg.NPC
        m = (row >= base) & (row < base + cfg.NPC)
        r = (row[m] - base).astype(np.int64)
        c = col[m].astype(np.int64)
        w = r >> 7
        cc = c // cfg.CHUNK
        seg_id = cc * cfg.W + w
        if getattr(cfg, "col_sort", False):
            # ascending cols inside each segment (HBM locality A/B knob)
            order = np.lexsort((c, seg_id))
        else:
            order = np.argsort(seg_id, kind="stable")
        r, c, seg_id = r[order], c[order], seg_id[order]
        counts = np.bincount(seg_id, minlength=cfg.C * cfg.W)
        if counts.size and r.size:
            maxc = max(maxc, int(counts.max()))
        per_core.append((r, c, seg_id, counts))
    SEG = int(math.ceil(maxc / 128.0)) * 128
    cfg.SEG = SEG
    EP = cfg.C * cfg.W * SEG

    in_maps = []
    for k in range(cfg.n_cores):
        r, c, seg_id, counts = per_core[k]
        starts = np.cumsum(counts) - counts
        intra = np.arange(r.size) - np.repeat(starts, counts)
        slots = seg_id * SEG + intra
        colidx = np.zeros(EP, np.int16)
        rowidx = np.zeros(EP, np.int16)
        rl = np.full(EP, 255.0, NP_BF16)
        colidx[slots] = (c - (c // cfg.CHUNK) * cfg.CHUNK).astype(np.int16)
        rowidx[slots] = r.astype(np.int16)
        rl[slots] = (r & 127).astype(NP_BF16)

        base = k * cfg.NPC
        hTown = np.ascontiguousarray(hTa[:, base:base + cfg.NPC])
        in_maps.append({
            "hTa": hTa,
            "hTown": hTown,
            "waug": waug,
            "nw1": nw1,
            "nw2a": nw2a,
            "iota": iota,
            "colidx": _wrap16(colidx),
            "rowidx": _wrap16(rowidx),
            "rl": _wrap128(rl),
        })
    return in_maps


def unshard_output(cfg, results):
    outs = [np.asarray(res["out"]) for res in results]
    full = np.concatenate(outs, axis=0)
    return np.ascontiguousarray(full[:cfg.N]).astype(np.float32)


# ---------------- entry point ----------------

def kernel(h, edge_index, edge_w, edge_b, node_w, node_b):
    from concourse.bass_utils import run_bass_kernel_spmd
    cfg = Cfg(n_nodes=100000, n_cores=8, spc=2)
    in_maps = prep_inputs(cfg, h, edge_index, edge_w, edge_b, node_w, node_b)
    nc = build_kernel(cfg)
    res = run_bass_kernel_spmd(nc, in_maps, core_ids=list(range(cfg.n_cores)))
    return unshard_output(cfg, res.results)

